# revision 1
# baseline (speedup 1.0000x reference)
"""Trainium2 Bass kernel for ClassWiseRegressionLoss.

reference semantics:
    idx = labels - 1                       # [N] in [0, C)
    class_pred[i] = pred[i, idx[i], :]     # [N, 2] gather
    d = class_pred - targets               # [N, 2]
    smooth_l1 = where(|d| < 1, 0.5 d^2, |d| - 0.5)
    out = mean(smooth_l1) * 2              # scalar f32

Strategy (data-parallel over N across 8 cores):
  pred is 400 MB but only 2 of 400 floats per row are used. Each core
  computes flat gather indices from its label shard on-device (DVE add
  against a constant base table), then fetches exactly the needed
  8-byte pairs from HBM with per-partition SWDGE indirect DMA gathers
  (the HW consumes one offset per partition per instruction, so K=256
  gather instructions cover 128x256 proposals). Smooth-L1 uses the
  abs-free split  sum(smooth) = 0.5*(sum(d^2) - sum(r1^2) - sum(m2^2))
  with r1 = max(d-1,0), m2 = min(d+1,0), which maps onto dual-op DVE
  tensor_scalar and ACT Square-with-accumulate ops. Each core returns
  [128, 3] per-partition component sums; the host reduces them in
  float64 and scales by 1/N.
"""

import functools

import numpy as np

import concourse.bacc as bacc
import concourse.bass as bass
import concourse.mybir as mybir
import concourse.tile as tile
from concourse.bass import IndirectOffsetOnAxis
from concourse.bass_utils import run_bass_kernel_spmd

N = 262144
C = 200
NCORES = 8
NLOC = N // NCORES  # 32768 proposals per core
P = 128  # SBUF partitions
K = NLOC // P  # 256 proposals per partition
NCHUNK = 4  # gather pipeline depth
KC = K // NCHUNK  # proposals per partition per chunk

f32 = mybir.dt.float32
i32 = mybir.dt.int32


@functools.lru_cache(maxsize=1)
def _build(nchunk: int = NCHUNK):
    kc = K // nchunk
    nc = bacc.Bacc(None, target_bir_lowering=False, debug=False)

    pred_t = nc.declare_dram_parameter("pred", [NLOC * C, 2], f32, isOutput=False)
    lab_t = nc.declare_dram_parameter("labels", [NLOC], i32, isOutput=False)
    base_t = nc.declare_dram_parameter("base", [NLOC], i32, isOutput=False)
    tgt_t = nc.declare_dram_parameter("targets", [NLOC, 2], f32, isOutput=False)
    # per-partition component sums: col 0 = sum(d^2), 1 = sum(u^2), 2 = sum(u)
    out_t = nc.declare_dram_parameter("partial", [P, 3], f32, isOutput=True)

    with tile.TileContext(nc) as tc:
        with (
            tc.tile_pool(name="io", bufs=1) as io,
            tc.tile_pool(name="work", bufs=3) as work,
        ):
            lab = io.tile([P, K], i32)
            nc.sync.dma_start(out=lab[:], in_=lab_t[:].rearrange("(p k) -> p k", p=P))
            basei = io.tile([P, K], i32)
            nc.sync.dma_start(
                out=basei[:], in_=base_t[:].rearrange("(p k) -> p k", p=P)
            )
            tg = io.tile([P, 2 * K], f32)
            nc.sync.dma_start(
                out=tg[:], in_=tgt_t[:].rearrange("(p k) two -> p (k two)", p=P)
            )

            # flat row index into pred[NLOC*C, 2]: n*C + label - 1
            idx = io.tile([P, K], i32)
            nc.vector.tensor_tensor(
                out=idx[:], in0=lab[:], in1=basei[:], op=mybir.AluOpType.add
            )
            # HW indirect DMA consumes ONE offset per partition per
            # instruction: gather the 2-float pair for one proposal per
            # partition at a time (proven tile_scatter_add form).
            g_all = io.tile([P, 2 * K], f32)
            for ci in range(K):
                nc.gpsimd.indirect_dma_start(
                    out=g_all[:, 2 * ci : 2 * ci + 2],
                    out_offset=None,
                    in_=pred_t[:, :],
                    in_offset=IndirectOffsetOnAxis(ap=idx[:, ci : ci + 1], axis=0),
                )

            d = io.tile([P, 2 * K], f32)
            nc.vector.tensor_tensor(
                out=d[:], in0=g_all[:], in1=tg[:], op=mybir.AluOpType.subtract
            )
            # relu(|d|-1)^2 = r1^2 + m2^2 with r1 = max(d-1,0), m2 = min(d+1,0)
            r1 = io.tile([P, 2 * K], f32)
            nc.vector.tensor_scalar(
                out=r1[:], in0=d[:], scalar1=-1.0, scalar2=0.0,
                op0=mybir.AluOpType.add, op1=mybir.AluOpType.max,
            )
            m2 = io.tile([P, 2 * K], f32)
            nc.vector.tensor_scalar(
                out=m2[:], in0=d[:], scalar1=1.0, scalar2=0.0,
                op0=mybir.AluOpType.add, op1=mybir.AluOpType.min,
            )
            part = io.tile([P, 3], f32)
            d2 = io.tile([P, 2 * K], f32)
            nc.scalar.activation(
                out=d2[:], in_=d[:],
                func=mybir.ActivationFunctionType.Square,
                accum_out=part[:, 0:1],
            )
            r12 = io.tile([P, 2 * K], f32)
            nc.scalar.activation(
                out=r12[:], in_=r1[:],
                func=mybir.ActivationFunctionType.Square,
                accum_out=part[:, 1:2],
            )
            m22 = io.tile([P, 2 * K], f32)
            nc.scalar.activation(
                out=m22[:], in_=m2[:],
                func=mybir.ActivationFunctionType.Square,
                accum_out=part[:, 2:3],
            )
            nc.sync.dma_start(out=out_t[:, :], in_=part[:])

    nc.compile()
    return nc


@functools.lru_cache(maxsize=1)
def _base_const() -> np.ndarray:
    # base[n] = n*C - 1 so that idx = base + label gives n*C + (label-1)
    return (np.arange(NLOC, dtype=np.int64) * C - 1).astype(np.int32)


def _run(pred, labels, targets, trace=False, nchunk: int = NCHUNK):
    pred = np.asarray(pred, dtype=np.float32)
    labels = np.asarray(labels)
    targets = np.asarray(targets, dtype=np.float32)
    assert pred.shape == (N, C, 2), pred.shape
    assert labels.shape == (N,), labels.shape
    assert targets.shape == (N, 2), targets.shape

    labels32 = labels.astype(np.int32, copy=False)
    base = _base_const()

    nc = _build(nchunk)
    in_maps = []
    for c in range(NCORES):
        sl = slice(c * NLOC, (c + 1) * NLOC)
        in_maps.append(
            {
                "pred": pred[sl].reshape(NLOC * C, 2),
                "labels": labels32[sl],
                "base": base,
                "targets": targets[sl],
            }
        )
    res = run_bass_kernel_spmd(nc, in_maps, list(range(NCORES)), trace=trace)
    total = 0.0
    for r in res.results:
        p = r["partial"].astype(np.float64)
        # sum(smooth_l1) = 0.5*(sum(d^2) - sum(r1^2) - sum(m2^2))
        total += 0.5 * (p[:, 0].sum() - p[:, 1].sum() - p[:, 2].sum())
    loss = np.float32(total / N)  # = mean * 2 over 2N elements
    return loss, res


def kernel(pred, labels, targets):
    loss, _ = _run(pred, labels, targets)
    return np.asarray(loss, dtype=np.float32)



# revision 4
# speedup vs baseline: 1.0433x; 1.0433x over previous
"""Trainium2 Bass kernel for ClassWiseRegressionLoss.

reference semantics:
    idx = labels - 1                       # [N] in [0, C)
    class_pred[i] = pred[i, idx[i], :]     # [N, 2] gather
    d = class_pred - targets               # [N, 2]
    smooth_l1 = where(|d| < 1, 0.5 d^2, |d| - 0.5)
    out = mean(smooth_l1) * 2              # scalar f32

Strategy (data-parallel over N across 8 cores):
  pred is 400 MB but only 2 of 400 floats per row are used. Each core
  computes flat gather indices from its label shard on-device (DVE add
  against a constant base table), then fetches exactly the needed
  8-byte pairs from HBM with per-partition SWDGE indirect DMA gathers
  (the HW consumes one offset per partition per instruction, so K=256
  gather instructions cover 128x256 proposals). Smooth-L1 uses the
  abs-free split  sum(smooth) = 0.5*(sum(d^2) - sum(r1^2) - sum(m2^2))
  with r1 = max(d-1,0), m2 = min(d+1,0), which maps onto dual-op DVE
  tensor_scalar and ACT Square-with-accumulate ops. Each core returns
  [128, 3] per-partition component sums; the host reduces them in
  float64 and scales by 1/N.
"""

import functools

import numpy as np

import concourse.bacc as bacc
import concourse.bass as bass
import concourse.mybir as mybir
import concourse.tile as tile
from concourse.bass import IndirectOffsetOnAxis
from concourse.bass_utils import run_bass_kernel_spmd

N = 262144
C = 200
NCORES = 8
NLOC = N // NCORES  # 32768 proposals per core
P = 128  # SBUF partitions
K = NLOC // P  # 256 proposals per partition
NCHUNK = 8  # number of multi-offset gather instructions (pipeline depth)

f32 = mybir.dt.float32
i32 = mybir.dt.int32


@functools.lru_cache(maxsize=1)
def _build(nchunk: int = NCHUNK):
    # nchunk multi-offset SWDGE gathers, each consuming K/nchunk offsets per
    # partition. SWDGE cost is ~994ns fixed per instruction + 0.34ns/descriptor,
    # so few big gathers beat many small ones by >10x.
    kc = K // nchunk
    assert nchunk * kc == K
    nc = bacc.Bacc(None, target_bir_lowering=False, debug=False)

    pred_t = nc.declare_dram_parameter("pred", [NLOC * C, 2], f32, isOutput=False)
    lab_t = nc.declare_dram_parameter("labels", [NLOC], i32, isOutput=False)
    base_t = nc.declare_dram_parameter("base", [NLOC], i32, isOutput=False)
    tgt_t = nc.declare_dram_parameter("targets", [NLOC, 2], f32, isOutput=False)
    # per-partition component sums: col 0 = sum(d^2), 1 = sum(u^2), 2 = sum(u)
    out_t = nc.declare_dram_parameter("partial", [P, 3], f32, isOutput=True)

    with tile.TileContext(nc) as tc:
        with (
            tc.tile_pool(name="io", bufs=1) as io,
            tc.tile_pool(name="work", bufs=3) as work,
        ):
            lab = io.tile([P, K], i32)
            nc.sync.dma_start(out=lab[:], in_=lab_t[:].rearrange("(p k) -> p k", p=P))
            basei = io.tile([P, K], i32)
            nc.sync.dma_start(
                out=basei[:], in_=base_t[:].rearrange("(p k) -> p k", p=P)
            )
            tg = io.tile([P, 2 * K], f32)
            nc.sync.dma_start(
                out=tg[:], in_=tgt_t[:].rearrange("(p k) two -> p (k two)", p=P)
            )

            # flat row index into pred[NLOC*C, 2]: n*C + label - 1
            idx = io.tile([P, K], i32)
            nc.vector.tensor_tensor(
                out=idx[:], in0=lab[:], in1=basei[:], op=mybir.AluOpType.add
            )
            # Multi-offset indirect gather: one instruction consumes kc offsets
            # per partition (P*kc descriptors, 8B each). out[p, 2j:2j+2] =
            # pred[idx[p, j], :].
            g_all = io.tile([P, 2 * K], f32)
            for ci in range(nchunk):
                c0 = ci * kc
                nc.gpsimd.indirect_dma_start(
                    out=g_all[:, 2 * c0 : 2 * (c0 + kc)],
                    out_offset=None,
                    in_=pred_t[:, :],
                    in_offset=IndirectOffsetOnAxis(ap=idx[:, c0 : c0 + kc], axis=0),
                )

            d = io.tile([P, 2 * K], f32)
            nc.vector.tensor_tensor(
                out=d[:], in0=g_all[:], in1=tg[:], op=mybir.AluOpType.subtract
            )
            # relu(|d|-1)^2 = r1^2 + m2^2 with r1 = max(d-1,0), m2 = min(d+1,0)
            r1 = io.tile([P, 2 * K], f32)
            nc.vector.tensor_scalar(
                out=r1[:], in0=d[:], scalar1=-1.0, scalar2=0.0,
                op0=mybir.AluOpType.add, op1=mybir.AluOpType.max,
            )
            m2 = io.tile([P, 2 * K], f32)
            nc.vector.tensor_scalar(
                out=m2[:], in0=d[:], scalar1=1.0, scalar2=0.0,
                op0=mybir.AluOpType.add, op1=mybir.AluOpType.min,
            )
            part = io.tile([P, 3], f32)
            d2 = io.tile([P, 2 * K], f32)
            nc.scalar.activation(
                out=d2[:], in_=d[:],
                func=mybir.ActivationFunctionType.Square,
                accum_out=part[:, 0:1],
            )
            r12 = io.tile([P, 2 * K], f32)
            nc.scalar.activation(
                out=r12[:], in_=r1[:],
                func=mybir.ActivationFunctionType.Square,
                accum_out=part[:, 1:2],
            )
            m22 = io.tile([P, 2 * K], f32)
            nc.scalar.activation(
                out=m22[:], in_=m2[:],
                func=mybir.ActivationFunctionType.Square,
                accum_out=part[:, 2:3],
            )
            nc.sync.dma_start(out=out_t[:, :], in_=part[:])

    nc.compile()
    return nc


@functools.lru_cache(maxsize=1)
def _base_const() -> np.ndarray:
    # base[n] = n*C - 1 so that idx = base + label gives n*C + (label-1)
    return (np.arange(NLOC, dtype=np.int64) * C - 1).astype(np.int32)


def _run(pred, labels, targets, trace=False, nchunk: int = NCHUNK):
    pred = np.asarray(pred, dtype=np.float32)
    labels = np.asarray(labels)
    targets = np.asarray(targets, dtype=np.float32)
    assert pred.shape == (N, C, 2), pred.shape
    assert labels.shape == (N,), labels.shape
    assert targets.shape == (N, 2), targets.shape

    labels32 = labels.astype(np.int32, copy=False)
    base = _base_const()

    nc = _build(nchunk)
    in_maps = []
    for c in range(NCORES):
        sl = slice(c * NLOC, (c + 1) * NLOC)
        in_maps.append(
            {
                "pred": pred[sl].reshape(NLOC * C, 2),
                "labels": labels32[sl],
                "base": base,
                "targets": targets[sl],
            }
        )
    res = run_bass_kernel_spmd(nc, in_maps, list(range(NCORES)), trace=trace)
    total = 0.0
    for r in res.results:
        p = r["partial"].astype(np.float64)
        # sum(smooth_l1) = 0.5*(sum(d^2) - sum(r1^2) - sum(m2^2))
        total += 0.5 * (p[:, 0].sum() - p[:, 1].sum() - p[:, 2].sum())
    loss = np.float32(total / N)  # = mean * 2 over 2N elements
    return loss, res


def kernel(pred, labels, targets):
    loss, _ = _run(pred, labels, targets)
    return np.asarray(loss, dtype=np.float32)

